# revision 8
# baseline (speedup 1.0000x reference)
"""Single-head causal attention on 8 NeuronCores (batch-parallel).

x [8, 2048, 1024], Wq/Wk/Wv [1024, 64] -> out [8, 2048, 64].
Each core handles one batch element.

v5: host-side layout prep + fully-overlapped schedule.
  - x uploaded pre-transposed and chunk-major ([P, NCH, NC, CH] bf16) so
    every DMA descriptor is a 4-8KB contiguous run; no on-chip x
    transposes.  [Wq|Wk] / Wv host-packed per c-tile.
  - All PE operands bf16 (FWL weight loads), f32 PSUM accumulation.
  - Scores run pairwise in the two 64-row PE groups concurrently
    (kq = [kT|qT] partition-swapped copy feeds the upper group).
    Diagonal tiles pair up the same way.
  - Attention runs a depth-2 software pipeline (scores lead PV by two
    groups) and projections for chunk ch+1 are interleaved into chunk
    ch's attention loop, so neither PE nor ACT ever waits long.
  - Output is normalized in [h, t] layout (reciprocal row + partition
    broadcast) and stored transposed; the host transposes it back.
  qkT = [Wq|Wk].T @ xT      (per 512-col chunk; q rows 0:64, k 64:128)
  weiT[s,t] = k[s]·q[t]; pT = exp(weiT/8)   (no max-subtraction)
  outT_aug = [v|1|0pad].T @ pT  (ones column gives softmax denominators)
  outT[h,t] = outT_aug[h,t] / outT_aug[64,t]
"""

from contextlib import ExitStack

import numpy as np
import ml_dtypes

import concourse.bass as bass
import concourse.mybir as mybir
import concourse.tile as tile
from concourse import bacc
from concourse.bass_utils import run_bass_kernel_spmd
from concourse.masks import make_identity, make_upper_triangular

B, T, C, H = 8, 2048, 1024, 64
P = 128                      # partition tile
NT = T // P                  # 16 row tiles
NC = C // P                  # 8 contraction tiles
CH = 512                     # t-chunk width (psum bank)
NCH = T // CH                # 4 chunks
TPC = CH // P                # 4 t-tiles per chunk
VP = 128                     # padded [v | 1 | 0] width (FWL needs 128 cols)

BF16 = mybir.dt.bfloat16
F32 = mybir.dt.float32

Exp = mybir.ActivationFunctionType.Exp

BF16_NP = ml_dtypes.bfloat16


def build_kernel():
    nc = bacc.Bacc(
        "TRN2",
        target_bir_lowering=False,
        debug=False,
        enable_asserts=False,
        num_devices=B,
    )
    xtd = nc.dram_tensor("xT", [P, NCH, NC, CH], BF16, kind="ExternalInput").ap()
    wqkd = nc.dram_tensor("wqk", [P, NC, P], BF16, kind="ExternalInput").ap()
    wvd = nc.dram_tensor("wv", [P, NC, H], BF16, kind="ExternalInput").ap()
    outd = nc.dram_tensor("out", [H, T], F32, kind="ExternalOutput").ap()

    with tile.TileContext(nc) as tc, ExitStack() as ctx:
        const = ctx.enter_context(tc.tile_pool(name="const", bufs=1))
        persist = ctx.enter_context(tc.tile_pool(name="persist", bufs=1))
        vtmp_p = ctx.enter_context(tc.tile_pool(name="vtmp", bufs=2))
        pt_p = ctx.enter_context(tc.tile_pool(name="pt", bufs=4))
        rc_p = ctx.enter_context(tc.tile_pool(name="rc", bufs=2))
        rcb_p = ctx.enter_context(tc.tile_pool(name="rcb", bufs=2))
        on_p = ctx.enter_context(tc.tile_pool(name="on", bufs=2))
        # PSUM: scratch (proj accum + v transposes) 2 banks,
        # wei 2x[128,1024] = 4 banks, o 2x[128,512] = 2 banks -> 8 total
        scr_ps = ctx.enter_context(tc.tile_pool(name="scrps", bufs=2, space="PSUM"))
        wei_ps = ctx.enter_context(tc.tile_pool(name="weips", bufs=2, space="PSUM"))
        o_ps_p = ctx.enter_context(tc.tile_pool(name="ops", bufs=2, space="PSUM"))

        # weights (vector queue) and x chunk 0 in halves (sync + scalar
        # queues) so descriptor generation is parallel; rest of x follows
        wqk = const.tile([P, NC, P], BF16, tag="wqk")
        nc.gpsimd.dma_start(wqk, wqkd)
        wv = const.tile([P, NC, H], BF16, tag="wv")
        nc.gpsimd.dma_start(wv, wvd)
        xTs = persist.tile([P, NCH, NC, CH], BF16, tag="xTs")  # x.T per chunk
        nc.sync.dma_start(xTs[:, 0, 0:4], xtd[:, 0, 0:4])
        nc.scalar.dma_start(xTs[:, 0, 4:NC], xtd[:, 0, 4:NC])
        for _c in range(1, NCH):
            nc.sync.dma_start(xTs[:, _c], xtd[:, _c])

        # masks: gpsimd builders write f32; DVE copy casts to bf16
        scr_i = const.tile([P, P], F32, tag="scr_i")
        make_identity(nc, scr_i)
        ident = const.tile([P, P], BF16, tag="ident")
        nc.vector.tensor_copy(ident, scr_i)
        scr_t = const.tile([P, P], F32, tag="scr_t")
        make_upper_triangular(nc, scr_t, val=1.0, diag=True)
        tri = const.tile([P, P], BF16, tag="tri")  # tri[p,j]=1 iff j>=p
        nc.vector.tensor_copy(tri, scr_t)

        qkT = persist.tile([P, T], BF16, tag="qkT")  # qT rows 0:64, kT 64:128
        kq = persist.tile([P, T], BF16, tag="kq")  # kT rows 0:64, qT 64:128
        vaug = persist.tile([P, NT, VP], BF16, tag="vaug")  # [v | 1 | 0pad]
        ones = nc.const_aps.scalar_like(1.0, vaug)
        nc.vector.tensor_copy(vaug[:, :, H : H + 1], ones.broadcast_to((P, NT, 1)))
        zeros = nc.const_aps.scalar_like(0.0, vaug)
        nc.vector.tensor_copy(
            vaug[:, :, H + 1 : VP], zeros.broadcast_to((P, NT, VP - H - 1))
        )

        def proj_ops(ch):
            """Projection thunks for chunk ch, split into an early part
            (projections, needed before chunk ch's attention) and a late
            part (v transposes, needed only by its diagonal PVs)."""
            chs = slice(ch * CH, (ch + 1) * CH)
            state = {}

            def qk_mm(c):
                def f():
                    if c == 0:
                        state["qk"] = scr_ps.tile([P, CH], F32, tag="scr", name="qk_ps")
                    nc.tensor.matmul(
                        state["qk"], wqk[:, c, :], xTs[:, ch, c, :],
                        start=(c == 0), stop=(c == NC - 1),
                    )
                return f

            def qk_out():
                nc.vector.tensor_copy(qkT[:, chs], state["qk"])
                nc.sync.dma_start(kq[0:H, chs], qkT[H:P, chs])
                nc.sync.dma_start(kq[H:P, chs], qkT[0:H, chs])

            def v_mm(c):
                def f():
                    if c == 0:
                        state["v"] = scr_ps.tile([P, CH], F32, tag="scr", name="v_ps")
                    nc.tensor.matmul(
                        state["v"][0:H, :], wv[:, c, :], xTs[:, ch, c, :],
                        start=(c == 0), stop=(c == NC - 1),
                    )
                return f

            def v_out():
                vtmp = vtmp_p.tile([H, CH], BF16)
                nc.vector.tensor_copy(vtmp, state["v"][0:H, :])
                state["vtmp"] = vtmp

            def v_tp(j):
                def f():
                    if j == 0:
                        state["vt"] = scr_ps.tile(
                            [P, TPC * H], BF16, tag="scr", name="vt_ps"
                        )
                    nc.tensor.transpose(
                        state["vt"][:, j * H : (j + 1) * H],
                        state["vtmp"][:, j * P : (j + 1) * P],
                        ident[0:H, 0:H],
                    )
                return f

            def v_aug():
                nc.vector.tensor_copy(
                    vaug[:, TPC * ch : TPC * ch + TPC, 0:H],
                    state["vt"].rearrange("p (j h) -> p j h", j=TPC),
                )

            early = [qk_mm(c) for c in range(NC)]
            early.append(qk_out)
            early += [v_mm(c) for c in range(NC)]
            early.append(v_out)
            late = [v_tp(j) for j in range(TPC)]
            late.append(v_aug)
            return early, late

        def run_ops(ops, n):
            for _ in range(n):
                if ops:
                    ops.pop(0)()

        # chunk 0 projections run up-front; its v-transposes go into the
        # head of chunk 0's attention loop
        early0, late_prev = proj_ops(0)
        for op in early0:
            op()

        scale = float(H) ** -0.5

        for ch in range(NCH):
            chs0 = ch * CH
            if ch + 1 < NCH:
                early_n, late_n = proj_ops(ch + 1)
            else:
                early_n, late_n = [], []
            inject = late_prev + early_n
            late_prev = late_n

            # attention groups: off-diag s-pairs, then 2 diagonal pairs
            groups = [("pair", 2 * i) for i in range(2 * ch)]
            groups.append(("dp1", TPC * ch))
            groups.append(("dp2", TPC * ch + 2))
            per_iter = -(-len(inject) // len(groups))  # ceil

            o_ps = o_ps_p.tile([P, CH], F32, tag="o")
            pipe = []  # (kind, sfirst, pT) awaiting PV
            first_pv = [True]

            def emit_pv(entry, stop, o_ps=o_ps, first_pv=first_pv):
                kind, s0, pT = entry
                st = first_pv[0]
                first_pv[0] = False
                if kind == "pair":
                    nc.tensor.matmul(
                        o_ps, vaug[:, s0, :], pT[:, 0:CH], start=st, stop=False
                    )
                    nc.tensor.matmul(
                        o_ps, vaug[:, s0 + 1, :], pT[:, CH : 2 * CH],
                        start=False, stop=stop,
                    )
                elif kind == "dp1":
                    nc.tensor.matmul(
                        o_ps, vaug[:, s0, :], pT[:, 0:CH], start=st, stop=False
                    )
                    nc.tensor.matmul(
                        o_ps[:, P:], vaug[:, s0 + 1, :], pT[:, CH : CH + 384],
                        start=False, stop=stop,
                    )
                else:  # dp2
                    nc.tensor.matmul(
                        o_ps[:, 2 * P :], vaug[:, s0, :], pT[:, 0:256],
                        start=st, stop=False,
                    )
                    nc.tensor.matmul(
                        o_ps[:, 3 * P :], vaug[:, s0 + 1, :], pT[:, CH : CH + P],
                        start=False, stop=stop,
                    )

            for kind, s0 in groups:
                wei = wei_ps.tile([P, 2 * CH], F32, tag="wei")
                s1 = s0 + 1
                if kind == "pair":
                    awid, boff, bwid = CH, 0, CH
                elif kind == "dp1":
                    awid, boff, bwid = CH, P, 384
                else:
                    awid, boff, bwid = 256, 3 * P, P
                # lower PE row group (kT at partitions 0:63)
                nc.tensor.matmul(
                    wei[:, 0:awid],
                    kq[0:H, s0 * P : (s0 + 1) * P],
                    qkT[0:H, chs0 + CH - awid : chs0 + CH],
                    start=True, stop=True,
                )
                # upper PE row group (kT rows of qkT, qT rows of kq)
                nc.tensor.matmul(
                    wei[:, CH : CH + bwid],
                    qkT[H:P, s1 * P : (s1 + 1) * P],
                    kq[H:P, chs0 + boff : chs0 + CH],
                    start=True, stop=True,
                )
                pT = pt_p.tile([P, 2 * CH], BF16)
                nc.scalar.activation(
                    pT[:, 0 : CH + bwid], wei[:, 0 : CH + bwid], Exp, scale=scale
                )
                if kind != "pair":
                    nc.vector.tensor_mul(pT[:, 0:P], pT[:, 0:P], tri)
                    nc.vector.tensor_mul(pT[:, CH : CH + P], pT[:, CH : CH + P], tri)
                pipe.append((kind, s0, pT))
                if len(pipe) > 2:
                    emit_pv(pipe.pop(0), stop=False)
                run_ops(inject, per_iter)

            run_ops(inject, len(inject))
            while pipe:
                entry = pipe.pop(0)
                emit_pv(entry, stop=(len(pipe) == 0))

            # ---- epilogue: normalize in [h, t] layout, store transposed ----
            rcr = rc_p.tile([1, CH], F32)
            nc.vector.reciprocal(rcr, o_ps[H : H + 1, :])
            rcb = rcb_p.tile([H, CH], F32)
            nc.gpsimd.partition_broadcast(rcb, rcr)
            onorm = on_p.tile([H, CH], F32)
            nc.vector.tensor_mul(onorm, o_ps[0:H, :], rcb)
            nc.sync.dma_start(outd[:, chs0 : chs0 + CH], onorm)

    nc.compile()
    return nc


_NC = None


def _pack_weights(Wq, Wk, Wv):
    # [C, H] -> stationary tiles [P, NC, ...]: wqk[p, c, 0:64]=Wq[c*128+p],
    # wqk[p, c, 64:128]=Wk[c*128+p]; wv[p, c, :]=Wv[c*128+p]
    wq = Wq.reshape(NC, P, H)
    wk = Wk.reshape(NC, P, H)
    wqk = np.concatenate([wq, wk], axis=2).transpose(1, 0, 2)  # [P, NC, 128]
    wv = Wv.reshape(NC, P, H).transpose(1, 0, 2)  # [P, NC, 64]
    return (
        np.ascontiguousarray(wqk).astype(BF16_NP),
        np.ascontiguousarray(wv).astype(BF16_NP),
    )


def kernel(x, Wq, Wk, Wv, **run_kwargs):
    global _NC
    if _NC is None:
        _NC = build_kernel()
    x = np.asarray(x, dtype=np.float32)
    Wq = np.asarray(Wq, dtype=np.float32)
    Wk = np.asarray(Wk, dtype=np.float32)
    Wv = np.asarray(Wv, dtype=np.float32)
    wqk, wv = _pack_weights(Wq, Wk, Wv)
    # [B, C, T] -> chunk-major [B, P, NCH, NC, CH]
    xT = x.transpose(0, 2, 1).astype(BF16_NP)
    xTq = np.ascontiguousarray(
        xT.reshape(B, NC, P, NCH, CH).transpose(0, 2, 3, 1, 4)
    )
    in_maps = [{"xT": xTq[b], "wqk": wqk, "wv": wv} for b in range(B)]
    res = run_bass_kernel_spmd(_NC, in_maps, core_ids=list(range(B)), **run_kwargs)
    out = np.stack([res.results[b]["out"].T for b in range(B)])
    if run_kwargs:
        kernel.last_result = res
    return out


if __name__ == "__main__":
    rng = np.random.default_rng(0)
    ins = {
        "x": rng.standard_normal((B, T, C), dtype=np.float32),
        "Wq": rng.standard_normal((C, H), dtype=np.float32) / np.sqrt(C),
        "Wk": rng.standard_normal((C, H), dtype=np.float32) / np.sqrt(C),
        "Wv": rng.standard_normal((C, H), dtype=np.float32) / np.sqrt(C),
    }
    out = kernel(**ins)
    print("out", out.shape, out.dtype)


# revision 9
# speedup vs baseline: 1.0148x; 1.0148x over previous
"""Single-head causal attention on 8 NeuronCores (batch-parallel).

x [8, 2048, 1024], Wq/Wk/Wv [1024, 64] -> out [8, 2048, 64].
Each core handles one batch element.

v5: host-side layout prep + fully-overlapped schedule.
  - x uploaded pre-transposed and chunk-major ([P, NCH, NC, CH] bf16) so
    every DMA descriptor is a 4-8KB contiguous run; no on-chip x
    transposes.  [Wq|Wk] / Wv host-packed per c-tile.
  - All PE operands bf16 (FWL weight loads), f32 PSUM accumulation.
  - Scores run pairwise in the two 64-row PE groups concurrently
    (kq = [kT|qT] partition-swapped copy feeds the upper group).
    Diagonal tiles pair up the same way.
  - Attention runs a depth-2 software pipeline (scores lead PV by two
    groups) and projections for chunk ch+1 are interleaved into chunk
    ch's attention loop, so neither PE nor ACT ever waits long.
  - Output is normalized in [h, t] layout (reciprocal row + partition
    broadcast) and stored transposed; the host transposes it back.
  qkT = [Wq|Wk].T @ xT      (per 512-col chunk; q rows 0:64, k 64:128)
  weiT[s,t] = k[s]·q[t]; pT = exp(weiT/8)   (no max-subtraction)
  outT_aug = [v|1|0pad].T @ pT  (ones column gives softmax denominators)
  outT[h,t] = outT_aug[h,t] / outT_aug[64,t]
"""

from contextlib import ExitStack

import numpy as np
import ml_dtypes

import concourse.bass as bass
import concourse.mybir as mybir
import concourse.tile as tile
from concourse import bacc
from concourse.bass_utils import run_bass_kernel_spmd
from concourse.masks import make_identity, make_upper_triangular

B, T, C, H = 8, 2048, 1024, 64
P = 128                      # partition tile
NT = T // P                  # 16 row tiles
NC = C // P                  # 8 contraction tiles
CH = 512                     # t-chunk width (psum bank)
NCH = T // CH                # 4 chunks
TPC = CH // P                # 4 t-tiles per chunk
VA = 96                      # [v | 1 | 0] width for the output transpose
VP = 128                     # padded [v | 1 | 0] width (FWL needs 128 cols)

BF16 = mybir.dt.bfloat16
F32 = mybir.dt.float32

Exp = mybir.ActivationFunctionType.Exp

BF16_NP = ml_dtypes.bfloat16


def build_kernel():
    nc = bacc.Bacc(
        "TRN2",
        target_bir_lowering=False,
        debug=False,
        enable_asserts=False,
        num_devices=B,
    )
    xtd = nc.dram_tensor("xT", [P, NCH, NC, CH], BF16, kind="ExternalInput").ap()
    wqkd = nc.dram_tensor("wqk", [P, NC, P], BF16, kind="ExternalInput").ap()
    wvd = nc.dram_tensor("wv", [P, NC, H], BF16, kind="ExternalInput").ap()
    outd = nc.dram_tensor("out", [T, H], F32, kind="ExternalOutput").ap()

    with tile.TileContext(nc) as tc, ExitStack() as ctx:
        const = ctx.enter_context(tc.tile_pool(name="const", bufs=1))
        persist = ctx.enter_context(tc.tile_pool(name="persist", bufs=1))
        vtmp_p = ctx.enter_context(tc.tile_pool(name="vtmp", bufs=2))
        pt_p = ctx.enter_context(tc.tile_pool(name="pt", bufs=4))
        rc_p = ctx.enter_context(tc.tile_pool(name="rc", bufs=2))
        osb_p = ctx.enter_context(tc.tile_pool(name="osb", bufs=2))
        ost_p = ctx.enter_context(tc.tile_pool(name="ost", bufs=2))
        # PSUM: scratch (proj accum + v transposes) 2 banks,
        # wei 2x[128,1024] = 4 banks, o 2x[128,512] = 2 banks -> 8 total
        scr_ps = ctx.enter_context(tc.tile_pool(name="scrps", bufs=2, space="PSUM"))
        wei_ps = ctx.enter_context(tc.tile_pool(name="weips", bufs=2, space="PSUM"))
        o_ps_p = ctx.enter_context(tc.tile_pool(name="ops", bufs=2, space="PSUM"))

        # weights (vector queue) and x chunk 0 in halves (sync + scalar
        # queues) so descriptor generation is parallel; rest of x follows
        xTs = persist.tile([P, NCH, NC, CH], BF16, tag="xTs")  # x.T per chunk
        nc.sync.dma_start(xTs[:, 0, 0:4], xtd[:, 0, 0:4])
        nc.gpsimd.dma_start(xTs[:, 0, 4:NC], xtd[:, 0, 4:NC])
        wqk = const.tile([P, NC, P], BF16, tag="wqk")
        nc.gpsimd.dma_start(wqk, wqkd)
        wv = const.tile([P, NC, H], BF16, tag="wv")
        nc.gpsimd.dma_start(wv, wvd)
        for _c in range(1, NCH):
            nc.sync.dma_start(xTs[:, _c], xtd[:, _c])

        # masks: gpsimd builders write f32; DVE copy casts to bf16
        scr_i = const.tile([P, P], F32, tag="scr_i")
        make_identity(nc, scr_i)
        ident = const.tile([P, P], BF16, tag="ident")
        nc.vector.tensor_copy(ident, scr_i)
        scr_t = const.tile([P, P], F32, tag="scr_t")
        make_upper_triangular(nc, scr_t, val=1.0, diag=True)
        tri = const.tile([P, P], BF16, tag="tri")  # tri[p,j]=1 iff j>=p
        nc.vector.tensor_copy(tri, scr_t)

        qkT = persist.tile([P, T], BF16, tag="qkT")  # qT rows 0:64, kT 64:128
        kq = persist.tile([P, T], BF16, tag="kq")  # kT rows 0:64, qT 64:128
        vaug = persist.tile([P, NT, VP], BF16, tag="vaug")  # [v | 1 | 0pad]
        ones = nc.const_aps.scalar_like(1.0, vaug)
        nc.vector.tensor_copy(vaug[:, :, H : H + 1], ones.broadcast_to((P, NT, 1)))
        zeros = nc.const_aps.scalar_like(0.0, vaug)
        nc.vector.tensor_copy(
            vaug[:, :, H + 1 : VP], zeros.broadcast_to((P, NT, VP - H - 1))
        )

        def proj_ops(ch):
            """Projection thunks for chunk ch, split into an early part
            (projections, needed before chunk ch's attention) and a late
            part (v transposes, needed only by its diagonal PVs)."""
            chs = slice(ch * CH, (ch + 1) * CH)
            state = {}

            def qk_mm(c):
                def f():
                    if c == 0:
                        state["qk"] = scr_ps.tile([P, CH], F32, tag="scr", name="qk_ps")
                    nc.tensor.matmul(
                        state["qk"], wqk[:, c, :], xTs[:, ch, c, :],
                        start=(c == 0), stop=(c == NC - 1),
                    )
                return f

            def qk_out():
                nc.vector.tensor_copy(qkT[:, chs], state["qk"])
                nc.sync.dma_start(kq[0:H, chs], qkT[H:P, chs])
                nc.sync.dma_start(kq[H:P, chs], qkT[0:H, chs])

            def v_mm(c):
                def f():
                    if c == 0:
                        state["v"] = scr_ps.tile([P, CH], F32, tag="scr", name="v_ps")
                    nc.tensor.matmul(
                        state["v"][0:H, :], wv[:, c, :], xTs[:, ch, c, :],
                        start=(c == 0), stop=(c == NC - 1),
                    )
                return f

            def v_out():
                vtmp = vtmp_p.tile([H, CH], BF16)
                nc.vector.tensor_copy(vtmp, state["v"][0:H, :])
                state["vtmp"] = vtmp

            def v_tp(j):
                def f():
                    if j == 0:
                        state["vt"] = scr_ps.tile(
                            [P, TPC * H], BF16, tag="scr", name="vt_ps"
                        )
                    nc.tensor.transpose(
                        state["vt"][:, j * H : (j + 1) * H],
                        state["vtmp"][:, j * P : (j + 1) * P],
                        ident[0:H, 0:H],
                    )
                return f

            def v_aug():
                nc.vector.tensor_copy(
                    vaug[:, TPC * ch : TPC * ch + TPC, 0:H],
                    state["vt"].rearrange("p (j h) -> p j h", j=TPC),
                )

            early = [qk_mm(c) for c in range(NC)]
            early.append(qk_out)
            early += [v_mm(c) for c in range(NC)]
            early.append(v_out)
            late = [v_tp(j) for j in range(TPC)]
            late.append(v_aug)
            return early, late

        def run_ops(ops, n):
            for _ in range(n):
                if ops:
                    ops.pop(0)()

        # chunk 0 projections run up-front; its v-transposes go into the
        # head of chunk 0's attention loop
        early0, late_prev = proj_ops(0)
        for op in early0:
            op()

        scale = float(H) ** -0.5

        for ch in range(NCH):
            chs0 = ch * CH
            if ch + 1 < NCH:
                early_n, late_n = proj_ops(ch + 1)
            else:
                early_n, late_n = [], []
            inject = late_prev + early_n
            late_prev = late_n

            # attention groups: off-diag s-pairs, then 2 diagonal pairs
            groups = [("pair", 2 * i) for i in range(2 * ch)]
            groups.append(("dp1", TPC * ch))
            groups.append(("dp2", TPC * ch + 2))
            per_iter = -(-len(inject) // len(groups))  # ceil

            o_ps = o_ps_p.tile([P, CH], F32, tag="o")
            pipe = []  # (kind, sfirst, pT) awaiting PV
            first_pv = [True]

            def emit_pv(entry, stop, o_ps=o_ps, first_pv=first_pv):
                kind, s0, pT = entry
                st = first_pv[0]
                first_pv[0] = False
                if kind == "pair":
                    nc.tensor.matmul(
                        o_ps, vaug[:, s0, :], pT[:, 0:CH], start=st, stop=False
                    )
                    nc.tensor.matmul(
                        o_ps, vaug[:, s0 + 1, :], pT[:, CH : 2 * CH],
                        start=False, stop=stop,
                    )
                elif kind == "dp1":
                    nc.tensor.matmul(
                        o_ps, vaug[:, s0, :], pT[:, 0:CH], start=st, stop=False
                    )
                    nc.tensor.matmul(
                        o_ps[:, P:], vaug[:, s0 + 1, :], pT[:, CH : CH + 384],
                        start=False, stop=stop,
                    )
                else:  # dp2
                    nc.tensor.matmul(
                        o_ps[:, 2 * P :], vaug[:, s0, :], pT[:, 0:256],
                        start=st, stop=False,
                    )
                    nc.tensor.matmul(
                        o_ps[:, 3 * P :], vaug[:, s0 + 1, :], pT[:, CH : CH + P],
                        start=False, stop=stop,
                    )

            for kind, s0 in groups:
                wei = wei_ps.tile([P, 2 * CH], F32, tag="wei")
                s1 = s0 + 1
                if kind == "pair":
                    awid, boff, bwid = CH, 0, CH
                elif kind == "dp1":
                    awid, boff, bwid = CH, P, 384
                else:
                    awid, boff, bwid = 256, 3 * P, P
                # lower PE row group (kT at partitions 0:63)
                nc.tensor.matmul(
                    wei[:, 0:awid],
                    kq[0:H, s0 * P : (s0 + 1) * P],
                    qkT[0:H, chs0 + CH - awid : chs0 + CH],
                    start=True, stop=True,
                )
                # upper PE row group (kT rows of qkT, qT rows of kq)
                nc.tensor.matmul(
                    wei[:, CH : CH + bwid],
                    qkT[H:P, s1 * P : (s1 + 1) * P],
                    kq[H:P, chs0 + boff : chs0 + CH],
                    start=True, stop=True,
                )
                pT = pt_p.tile([P, 2 * CH], BF16)
                nc.scalar.activation(
                    pT[:, 0 : CH + bwid], wei[:, 0 : CH + bwid], Exp, scale=scale
                )
                if kind != "pair":
                    nc.vector.tensor_mul(pT[:, 0:P], pT[:, 0:P], tri)
                    nc.vector.tensor_mul(pT[:, CH : CH + P], pT[:, CH : CH + P], tri)
                pipe.append((kind, s0, pT))
                if len(pipe) > 2:
                    emit_pv(pipe.pop(0), stop=False)
                run_ops(inject, per_iter)

            run_ops(inject, len(inject))
            while pipe:
                entry = pipe.pop(0)
                emit_pv(entry, stop=(len(pipe) == 0))

            # ---- epilogue: transpose back, normalize, store ----
            osb = osb_p.tile([VA, CH], BF16)
            ot_ps = scr_ps.tile([P, TPC * VA], BF16, tag="scr")
            for j in range(TPC):
                nc.vector.tensor_copy(
                    osb[:, j * P : (j + 1) * P], o_ps[0:VA, j * P : (j + 1) * P]
                )
                nc.tensor.transpose(
                    ot_ps[:, j * VA : (j + 1) * VA],
                    osb[:, j * P : (j + 1) * P],
                    ident[0:VA, 0:VA],
                )
            otv = ot_ps.rearrange("p (j v) -> p j v", j=TPC)
            rc = rc_p.tile([P, TPC, 1], F32)
            nc.vector.reciprocal(rc, otv[:, :, H : H + 1])
            ost = ost_p.tile([P, TPC, H], F32)
            nc.vector.tensor_mul(ost, otv[:, :, 0:H], rc.broadcast_to((P, TPC, H)))
            nc.sync.dma_start(
                outd[chs0 : chs0 + CH, :].rearrange("(n p) h -> p n h", p=P), ost
            )

    nc.compile()
    return nc


_NC = None


def _pack_weights(Wq, Wk, Wv):
    # [C, H] -> stationary tiles [P, NC, ...]: wqk[p, c, 0:64]=Wq[c*128+p],
    # wqk[p, c, 64:128]=Wk[c*128+p]; wv[p, c, :]=Wv[c*128+p]
    wq = Wq.reshape(NC, P, H)
    wk = Wk.reshape(NC, P, H)
    wqk = np.concatenate([wq, wk], axis=2).transpose(1, 0, 2)  # [P, NC, 128]
    wv = Wv.reshape(NC, P, H).transpose(1, 0, 2)  # [P, NC, 64]
    return (
        np.ascontiguousarray(wqk).astype(BF16_NP),
        np.ascontiguousarray(wv).astype(BF16_NP),
    )


def kernel(x, Wq, Wk, Wv, **run_kwargs):
    global _NC
    if _NC is None:
        _NC = build_kernel()
    x = np.asarray(x, dtype=np.float32)
    Wq = np.asarray(Wq, dtype=np.float32)
    Wk = np.asarray(Wk, dtype=np.float32)
    Wv = np.asarray(Wv, dtype=np.float32)
    wqk, wv = _pack_weights(Wq, Wk, Wv)
    # [B, C, T] -> chunk-major [B, P, NCH, NC, CH]
    xT = x.transpose(0, 2, 1).astype(BF16_NP)
    xTq = np.ascontiguousarray(
        xT.reshape(B, NC, P, NCH, CH).transpose(0, 2, 3, 1, 4)
    )
    in_maps = [{"xT": xTq[b], "wqk": wqk, "wv": wv} for b in range(B)]
    res = run_bass_kernel_spmd(_NC, in_maps, core_ids=list(range(B)), **run_kwargs)
    out = np.stack([res.results[b]["out"] for b in range(B)])
    if run_kwargs:
        kernel.last_result = res
    return out


if __name__ == "__main__":
    rng = np.random.default_rng(0)
    ins = {
        "x": rng.standard_normal((B, T, C), dtype=np.float32),
        "Wq": rng.standard_normal((C, H), dtype=np.float32) / np.sqrt(C),
        "Wk": rng.standard_normal((C, H), dtype=np.float32) / np.sqrt(C),
        "Wv": rng.standard_normal((C, H), dtype=np.float32) / np.sqrt(C),
    }
    out = kernel(**ins)
    print("out", out.shape, out.dtype)


# revision 10
# speedup vs baseline: 1.0849x; 1.0691x over previous
"""Single-head causal attention on 8 NeuronCores (batch-parallel).

x [8, 2048, 1024], Wq/Wk/Wv [1024, 64] -> out [8, 2048, 64].
Each core handles one batch element.

v5: host-side layout prep + fully-overlapped schedule.
  - x uploaded pre-transposed and chunk-major ([P, NCH, NC, CH] bf16) so
    every DMA descriptor is a 4-8KB contiguous run; no on-chip x
    transposes.  [Wq|Wk] / Wv host-packed per c-tile.
  - All PE operands bf16 (FWL weight loads), f32 PSUM accumulation.
  - Scores run pairwise in the two 64-row PE groups concurrently
    (kq = [kT|qT] partition-swapped copy feeds the upper group).
    Diagonal tiles pair up the same way.
  - Attention runs a depth-2 software pipeline (scores lead PV by two
    groups) and projections for chunk ch+1 are interleaved into chunk
    ch's attention loop, so neither PE nor ACT ever waits long.
  - Output is normalized in [h, t] layout (reciprocal row + partition
    broadcast) and stored transposed; the host transposes it back.
  qkT = [Wq|Wk].T @ xT      (per 512-col chunk; q rows 0:64, k 64:128)
  weiT[s,t] = k[s]·q[t]; pT = exp(weiT/8)   (no max-subtraction)
  outT_aug = [v|1|0pad].T @ pT  (ones column gives softmax denominators)
  outT[h,t] = outT_aug[h,t] / outT_aug[64,t]
"""

from contextlib import ExitStack

import numpy as np
import ml_dtypes

import concourse.bass as bass
import concourse.mybir as mybir
import concourse.tile as tile
from concourse import bacc
from concourse.bass_utils import run_bass_kernel_spmd
from concourse.masks import make_identity, make_upper_triangular

B, T, C, H = 8, 2048, 1024, 64
P = 128                      # partition tile
NT = T // P                  # 16 row tiles
NC = C // P                  # 8 contraction tiles
CH = 512                     # t-chunk width (psum bank)
NCH = T // CH                # 4 chunks
TPC = CH // P                # 4 t-tiles per chunk
VA = 96                      # [v | 1 | 0] width for the output transpose
VP = 128                     # padded [v | 1 | 0] width (FWL needs 128 cols)

BF16 = mybir.dt.bfloat16
F32 = mybir.dt.float32

Exp = mybir.ActivationFunctionType.Exp

BF16_NP = ml_dtypes.bfloat16


def build_kernel():
    nc = bacc.Bacc(
        "TRN2",
        target_bir_lowering=False,
        debug=False,
        enable_asserts=False,
        num_devices=B,
    )
    xtd = nc.dram_tensor("xT", [P, NCH, NC, CH], BF16, kind="ExternalInput").ap()
    wqkd = nc.dram_tensor("wqk", [P, NC, P], BF16, kind="ExternalInput").ap()
    wvd = nc.dram_tensor("wv", [P, NC, H], BF16, kind="ExternalInput").ap()
    outd = nc.dram_tensor("out", [T, H], F32, kind="ExternalOutput").ap()

    with tile.TileContext(nc) as tc, ExitStack() as ctx:
        const = ctx.enter_context(tc.tile_pool(name="const", bufs=1))
        persist = ctx.enter_context(tc.tile_pool(name="persist", bufs=1))
        vtmp_p = ctx.enter_context(tc.tile_pool(name="vtmp", bufs=2))
        pt_p = ctx.enter_context(tc.tile_pool(name="pt", bufs=4))
        rc_p = ctx.enter_context(tc.tile_pool(name="rc", bufs=2))
        osb_p = ctx.enter_context(tc.tile_pool(name="osb", bufs=2))
        ost_p = ctx.enter_context(tc.tile_pool(name="ost", bufs=2))
        # PSUM: scratch (proj accum + v transposes) 2 banks,
        # wei 2x[128,1024] = 4 banks, o 2x[128,512] = 2 banks -> 8 total
        scr_ps = ctx.enter_context(tc.tile_pool(name="scrps", bufs=2, space="PSUM"))
        wei_ps = ctx.enter_context(tc.tile_pool(name="weips", bufs=2, space="PSUM"))
        o_ps_p = ctx.enter_context(tc.tile_pool(name="ops", bufs=2, space="PSUM"))

        # weights (vector queue) and x chunk 0 in halves (sync + scalar
        # queues) so descriptor generation is parallel; rest of x follows
        wqk = const.tile([P, NC, P], BF16, tag="wqk")
        nc.sync.dma_start(wqk, wqkd)
        wv = const.tile([P, NC, H], BF16, tag="wv")
        nc.sync.dma_start(wv, wvd)
        xTs = persist.tile([P, NCH, NC, CH], BF16, tag="xTs")  # x.T per chunk
        nc.sync.dma_start(xTs[:, 0, 0:4], xtd[:, 0, 0:4])
        nc.sync.dma_start(xTs[:, 0, 4:NC], xtd[:, 0, 4:NC])
        for _c in range(1, NCH):
            nc.sync.dma_start(xTs[:, _c], xtd[:, _c])

        # masks: gpsimd builders write f32; DVE copy casts to bf16
        scr_i = const.tile([P, P], F32, tag="scr_i")
        make_identity(nc, scr_i)
        ident = const.tile([P, P], BF16, tag="ident")
        nc.vector.tensor_copy(ident, scr_i)
        scr_t = const.tile([P, P], F32, tag="scr_t")
        make_upper_triangular(nc, scr_t, val=1.0, diag=True)
        tri = const.tile([P, P], BF16, tag="tri")  # tri[p,j]=1 iff j>=p
        nc.vector.tensor_copy(tri, scr_t)

        qkT = persist.tile([P, T], BF16, tag="qkT")  # qT rows 0:64, kT 64:128
        kq = persist.tile([P, T], BF16, tag="kq")  # kT rows 0:64, qT 64:128
        vaug = persist.tile([P, NT, VP], BF16, tag="vaug")  # [v | 1 | 0pad]
        ones = nc.const_aps.scalar_like(1.0, vaug)
        nc.vector.tensor_copy(vaug[:, :, H : H + 1], ones.broadcast_to((P, NT, 1)))
        zeros = nc.const_aps.scalar_like(0.0, vaug)
        nc.vector.tensor_copy(
            vaug[:, :, H + 1 : VP], zeros.broadcast_to((P, NT, VP - H - 1))
        )

        def proj_ops(ch):
            """Projection thunks for chunk ch, split into an early part
            (projections, needed before chunk ch's attention) and a late
            part (v transposes, needed only by its diagonal PVs)."""
            chs = slice(ch * CH, (ch + 1) * CH)
            state = {}

            def qk_mm(c):
                def f():
                    if c == 0:
                        state["qk"] = scr_ps.tile([P, CH], F32, tag="scr", name="qk_ps")
                    nc.tensor.matmul(
                        state["qk"], wqk[:, c, :], xTs[:, ch, c, :],
                        start=(c == 0), stop=(c == NC - 1),
                    )
                return f

            def qk_out():
                nc.vector.tensor_copy(qkT[:, chs], state["qk"])
                nc.sync.dma_start(kq[0:H, chs], qkT[H:P, chs])
                nc.sync.dma_start(kq[H:P, chs], qkT[0:H, chs])

            def v_mm(c):
                def f():
                    if c == 0:
                        state["v"] = scr_ps.tile([P, CH], F32, tag="scr", name="v_ps")
                    nc.tensor.matmul(
                        state["v"][0:H, :], wv[:, c, :], xTs[:, ch, c, :],
                        start=(c == 0), stop=(c == NC - 1),
                    )
                return f

            def v_out():
                vtmp = vtmp_p.tile([H, CH], BF16)
                nc.vector.tensor_copy(vtmp, state["v"][0:H, :])
                state["vtmp"] = vtmp

            def v_tp(j):
                def f():
                    if j == 0:
                        state["vt"] = scr_ps.tile(
                            [P, TPC * H], BF16, tag="scr", name="vt_ps"
                        )
                    nc.tensor.transpose(
                        state["vt"][:, j * H : (j + 1) * H],
                        state["vtmp"][:, j * P : (j + 1) * P],
                        ident[0:H, 0:H],
                    )
                return f

            def v_aug():
                nc.vector.tensor_copy(
                    vaug[:, TPC * ch : TPC * ch + TPC, 0:H],
                    state["vt"].rearrange("p (j h) -> p j h", j=TPC),
                )

            early = [qk_mm(c) for c in range(NC)]
            early.append(qk_out)
            early += [v_mm(c) for c in range(NC)]
            early.append(v_out)
            late = [v_tp(j) for j in range(TPC)]
            late.append(v_aug)
            return early, late

        def run_ops(ops, n):
            for _ in range(n):
                if ops:
                    ops.pop(0)()

        # chunk 0 projections run up-front; its v-transposes go into the
        # head of chunk 0's attention loop
        early0, late_prev = proj_ops(0)
        for op in early0:
            op()

        scale = float(H) ** -0.5

        for ch in range(NCH):
            chs0 = ch * CH
            if ch + 1 < NCH:
                early_n, late_n = proj_ops(ch + 1)
            else:
                early_n, late_n = [], []
            inject = late_prev + early_n
            late_prev = late_n

            # attention groups: off-diag s-pairs, then 2 diagonal pairs
            groups = [("pair", 2 * i) for i in range(2 * ch)]
            groups.append(("dp1", TPC * ch))
            groups.append(("dp2", TPC * ch + 2))
            per_iter = -(-len(inject) // len(groups))  # ceil

            o_ps = o_ps_p.tile([P, CH], F32, tag="o")
            pipe = []  # (kind, sfirst, pT) awaiting PV
            first_pv = [True]

            def emit_pv(entry, stop, o_ps=o_ps, first_pv=first_pv):
                kind, s0, pT = entry
                st = first_pv[0]
                first_pv[0] = False
                if kind == "pair":
                    nc.tensor.matmul(
                        o_ps, vaug[:, s0, :], pT[:, 0:CH], start=st, stop=False
                    )
                    nc.tensor.matmul(
                        o_ps, vaug[:, s0 + 1, :], pT[:, CH : 2 * CH],
                        start=False, stop=stop,
                    )
                elif kind == "dp1":
                    nc.tensor.matmul(
                        o_ps, vaug[:, s0, :], pT[:, 0:CH], start=st, stop=False
                    )
                    nc.tensor.matmul(
                        o_ps[:, P:], vaug[:, s0 + 1, :], pT[:, CH : CH + 384],
                        start=False, stop=stop,
                    )
                else:  # dp2
                    nc.tensor.matmul(
                        o_ps[:, 2 * P :], vaug[:, s0, :], pT[:, 0:256],
                        start=st, stop=False,
                    )
                    nc.tensor.matmul(
                        o_ps[:, 3 * P :], vaug[:, s0 + 1, :], pT[:, CH : CH + P],
                        start=False, stop=stop,
                    )

            for kind, s0 in groups:
                wei = wei_ps.tile([P, 2 * CH], F32, tag="wei")
                s1 = s0 + 1
                if kind == "pair":
                    awid, boff, bwid = CH, 0, CH
                elif kind == "dp1":
                    awid, boff, bwid = CH, P, 384
                else:
                    awid, boff, bwid = 256, 3 * P, P
                # lower PE row group (kT at partitions 0:63)
                nc.tensor.matmul(
                    wei[:, 0:awid],
                    kq[0:H, s0 * P : (s0 + 1) * P],
                    qkT[0:H, chs0 + CH - awid : chs0 + CH],
                    start=True, stop=True,
                )
                # upper PE row group (kT rows of qkT, qT rows of kq)
                nc.tensor.matmul(
                    wei[:, CH : CH + bwid],
                    qkT[H:P, s1 * P : (s1 + 1) * P],
                    kq[H:P, chs0 + boff : chs0 + CH],
                    start=True, stop=True,
                )
                pT = pt_p.tile([P, 2 * CH], BF16)
                nc.scalar.activation(
                    pT[:, 0 : CH + bwid], wei[:, 0 : CH + bwid], Exp, scale=scale
                )
                if kind != "pair":
                    nc.vector.tensor_mul(pT[:, 0:P], pT[:, 0:P], tri)
                    nc.vector.tensor_mul(pT[:, CH : CH + P], pT[:, CH : CH + P], tri)
                pipe.append((kind, s0, pT))
                if len(pipe) > 2:
                    emit_pv(pipe.pop(0), stop=False)
                run_ops(inject, per_iter)

            run_ops(inject, len(inject))
            while pipe:
                entry = pipe.pop(0)
                emit_pv(entry, stop=(len(pipe) == 0))

            # ---- epilogue: transpose back, normalize, store ----
            osb = osb_p.tile([VA, CH], BF16)
            ot_ps = scr_ps.tile([P, TPC * VA], BF16, tag="scr")
            for j in range(TPC):
                nc.vector.tensor_copy(
                    osb[:, j * P : (j + 1) * P], o_ps[0:VA, j * P : (j + 1) * P]
                )
                nc.tensor.transpose(
                    ot_ps[:, j * VA : (j + 1) * VA],
                    osb[:, j * P : (j + 1) * P],
                    ident[0:VA, 0:VA],
                )
            otv = ot_ps.rearrange("p (j v) -> p j v", j=TPC)
            rc = rc_p.tile([P, TPC, 1], F32)
            nc.vector.reciprocal(rc, otv[:, :, H : H + 1])
            ost = ost_p.tile([P, TPC, H], F32)
            nc.vector.tensor_mul(ost, otv[:, :, 0:H], rc.broadcast_to((P, TPC, H)))
            nc.sync.dma_start(
                outd[chs0 : chs0 + CH, :].rearrange("(n p) h -> p n h", p=P), ost
            )

    nc.compile()
    return nc


_NC = None


def _pack_weights(Wq, Wk, Wv):
    # [C, H] -> stationary tiles [P, NC, ...]: wqk[p, c, 0:64]=Wq[c*128+p],
    # wqk[p, c, 64:128]=Wk[c*128+p]; wv[p, c, :]=Wv[c*128+p]
    wq = Wq.reshape(NC, P, H)
    wk = Wk.reshape(NC, P, H)
    wqk = np.concatenate([wq, wk], axis=2).transpose(1, 0, 2)  # [P, NC, 128]
    wv = Wv.reshape(NC, P, H).transpose(1, 0, 2)  # [P, NC, 64]
    return (
        np.ascontiguousarray(wqk).astype(BF16_NP),
        np.ascontiguousarray(wv).astype(BF16_NP),
    )


def kernel(x, Wq, Wk, Wv, **run_kwargs):
    global _NC
    if _NC is None:
        _NC = build_kernel()
    x = np.asarray(x, dtype=np.float32)
    Wq = np.asarray(Wq, dtype=np.float32)
    Wk = np.asarray(Wk, dtype=np.float32)
    Wv = np.asarray(Wv, dtype=np.float32)
    wqk, wv = _pack_weights(Wq, Wk, Wv)
    # [B, C, T] -> chunk-major [B, P, NCH, NC, CH]
    xT = x.transpose(0, 2, 1).astype(BF16_NP)
    xTq = np.ascontiguousarray(
        xT.reshape(B, NC, P, NCH, CH).transpose(0, 2, 3, 1, 4)
    )
    in_maps = [{"xT": xTq[b], "wqk": wqk, "wv": wv} for b in range(B)]
    res = run_bass_kernel_spmd(_NC, in_maps, core_ids=list(range(B)), **run_kwargs)
    out = np.stack([res.results[b]["out"] for b in range(B)])
    if run_kwargs:
        kernel.last_result = res
    return out


if __name__ == "__main__":
    rng = np.random.default_rng(0)
    ins = {
        "x": rng.standard_normal((B, T, C), dtype=np.float32),
        "Wq": rng.standard_normal((C, H), dtype=np.float32) / np.sqrt(C),
        "Wk": rng.standard_normal((C, H), dtype=np.float32) / np.sqrt(C),
        "Wv": rng.standard_normal((C, H), dtype=np.float32) / np.sqrt(C),
    }
    out = kernel(**ins)
    print("out", out.shape, out.dtype)
